# revision 36
# baseline (speedup 1.0000x reference)
"""Trainium2 Bass kernel for nn_AttentionTanh (B=8, S=2048, F=1024, U=256).

Data-parallel over batch: each of the 8 NeuronCores computes the full
attention for one batch example. No collectives.

Per-core dataflow (all matmuls via TensorE, out = lhsT.T @ rhs):
  xT   [F, S]  (host-transposed input shard, F on partitions)
  QT   [u, s] = tanh(Wq.T @ x.T)  -> matmul(lhsT=Wq[f,u], rhs=xT[f,s])
  KT   [u, s] = tanh(Wk.T @ x.T)
  V    [s, u] = tanh(x @ Wv)      -> matmul(lhsT=xT[f,s], rhs=Wv[f,u])
                V gets a fused ones-column so the out-matmul also
                produces the softmax denominator (column U).
  eST  [t, q] = exp(scale * K q.T) -> matmul(lhsT=KT[u,t], rhs=QT[u,q])
                (tanh bounds scores to [-8, 8]; no max subtraction needed)
  out  [q, u] = (eST.T @ [V | 1]) row-normalized by its last column.
"""

import os
import sys

import numpy as np

for _p in ("/opt/trn_rl_repo", "/root/.axon_site/_ro/trn_rl_repo"):
    if os.path.isdir(_p) and _p not in sys.path:
        sys.path.append(_p)

import concourse.bass as bass
import concourse.mybir as mybir
import concourse.tile as tile
from concourse.bass_utils import run_bass_kernel_spmd

P = 128
B, S, F, U = 8, 2048, 1024, 256
FO, SO, UO = F // P, S // P, U // P  # 8, 16, 2
SB = 512                             # s-block width for DMA/projections
NSB = S // SB                        # 4
QB = 512                             # query-block width (free dim of eST)
NQB = S // QB                        # 4
SCALE = 1.0 / float(np.sqrt(F))      # 1/32
VW = U + 2                           # V plus fused ones columns (even width
                                     # keeps fp32r's free-count rules happy)
F32 = mybir.dt.float32

# Compute dtype for TensorE matmuls: "float32", "float32r", or "bfloat16".
CDT = "float32r"


def _split_matmul_waits(nc):
    """Walrus instruction structs have a single sem-wait slot (EventSemaphore
    has two). Peel excess waits onto NoOps (plain wait instructions on the
    same engine) inserted just before the overloaded instruction."""
    n = 0
    for bb in nc.m.functions[0].blocks:
        new_insts = []
        for inst in bb.instructions:
            cap = 2 if isinstance(inst, mybir.InstEventSemaphore) else 1
            if (
                inst.sync_info
                and inst.sync_info.on_wait
                and len(inst.sync_info.on_wait) > cap
            ):
                waits = list(inst.sync_info.on_wait)
                for w in waits[cap:]:
                    n += 1
                    nop = mybir.InstNoOp(name=f"I-xwait-{n}", ins=[], outs=[])
                    nop.engine = inst.engine
                    nop.sync_info = mybir.SyncInfo(on_wait=[w], on_update=[])
                    new_insts.append(nop)
                inst.sync_info.on_wait = waits[:cap]
            new_insts.append(inst)
        bb.instructions[:] = new_insts
    return n


def build_nc(cdt_name=CDT, split_waits=True):
    cdt = getattr(mybir.dt, cdt_name)
    store_dt = F32 if cdt == F32 else cdt
    # float32r shares the fp32 bit layout, so DRAM parameters can be declared
    # f32r directly and DMA'd without a rounding cast; bf16 still needs the
    # staged cast copy after DMA.
    in_dt = cdt if cdt == mybir.dt.float32r else F32
    needs_cast = store_dt != in_dt

    nc = bass.Bass()
    xT_d = nc.declare_dram_parameter("xT", [F, S], in_dt, isOutput=False)
    w_d = {
        k: nc.declare_dram_parameter(k, [F, U], in_dt, isOutput=False)
        for k in ("Wq", "Wk", "Wv")
    }
    out_d = nc.declare_dram_parameter("out", [S, U], F32, isOutput=True)

    TANH = mybir.ActivationFunctionType.Tanh
    EXP = mybir.ActivationFunctionType.Exp

    with tile.TileContext(nc) as tc:
        with (
            tc.tile_pool(name="wpool", bufs=1) as wpool,
            tc.tile_pool(name="xpool", bufs=1) as xpool,
            tc.tile_pool(name="xstage", bufs=2) as xstage,
            tc.tile_pool(name="qkv", bufs=1) as qkv,
            tc.tile_pool(
                name="exps", bufs=2 if store_dt == mybir.dt.bfloat16 else 1
            ) as exps,
            tc.tile_pool(name="smalls", bufs=1) as smalls,
            tc.tile_pool(name="recs", bufs=2) as recs,
            tc.tile_pool(name="evac", bufs=4) as evac,
            tc.tile_pool(name="ps_big", bufs=2, space="PSUM") as ps_big,
            tc.tile_pool(name="ps_v", bufs=2, space="PSUM") as ps_v,
            tc.tile_pool(name="ps_o", bufs=2, space="PSUM") as ps_o,
            tc.tile_pool(name="ps_d", bufs=1, space="PSUM") as ps_dp,
        ):
            # ---- input DMAs, spread across engine queues so they run in
            # parallel instead of serializing on one HWDGE ring ----
            # x s-blocks are the critical path for the first matmuls.
            xT = xpool.tile([P, FO, S], store_dt)
            xT_src = xT_d[:].rearrange("(fo fi) s -> fi fo s", fi=P)
            x_dma_engines = [nc.sync, nc.scalar, nc.gpsimd, nc.sync]
            x_stage = []
            for sb in range(NSB):
                sl = slice(sb * SB, (sb + 1) * SB)
                eng = x_dma_engines[sb % len(x_dma_engines)]
                if not needs_cast:
                    eng.dma_start(xT[:, :, sl], xT_src[:, :, sl])
                else:
                    xs = xstage.tile([P, FO, SB], in_dt, tag="stage")
                    eng.dma_start(xs[:], xT_src[:, :, sl])
                    x_stage.append((sl, xs))

            # ---- weights: [F, U] -> [fi=128, fo=8, u=256] (+ cast) ----
            # bf16 staging slots are shared with the x loads (same tag).
            w_t = {}
            w_dma_engines = {"Wq": nc.gpsimd, "Wk": nc.scalar, "Wv": nc.sync}
            for k in ("Wq", "Wk", "Wv"):
                if needs_cast:
                    wstg = xstage.tile(
                        [P, FO, SB], in_dt, tag="stage", name=f"wstg_{k}"
                    )
                    wf = wstg[:, :, :U]
                else:
                    wf = wpool.tile([P, FO, U], in_dt, tag=f"{k}_in")
                w_dma_engines[k].dma_start(
                    wf[:], w_d[k][:].rearrange("(fo fi) u -> fi fo u", fi=P)
                )
                if needs_cast:
                    wc = wpool.tile([P, FO, U], store_dt, tag=f"{k}_c")
                    nc.vector.tensor_copy(wc[:], wf[:])
                    w_t[k] = wc
                else:
                    w_t[k] = wf
            for sl, xs in x_stage:
                nc.vector.tensor_copy(xT[:, :, sl], xs[:])

            # ---- PE warmup: junk matmuls on a zeroed tile keep the PE busy
            # while the x DMAs land, so HAM un-throttles before real work ----
            warm_f32 = smalls.tile([P, SB], F32, tag="warm_f32")
            nc.vector.memset(warm_f32[:], 0.0)
            if store_dt == F32:
                warm = warm_f32
            else:
                warm = smalls.tile([P, SB], store_dt, tag="warm")
                nc.vector.tensor_copy(warm[:], warm_f32[:])
            ps_w = ps_dp.tile([P, SB], F32, tag="ps_w")
            for _ in range(12):
                nc.tensor.matmul(
                    ps_w[:], warm[:, :P], warm[:], start=True, stop=True
                )

            # ---- projections (per s-block so PE starts as DMA lands) ----
            qT = qkv.tile([P, UO, S], store_dt, tag="qT")
            kT = qkv.tile([P, UO, S], store_dt, tag="kT")
            # V gets two fused ones-columns: the out-matmul then also produces
            # the softmax denominator (cols U:U+2; two columns keep fp32r's
            # even-free-count rule satisfied).
            vv = qkv.tile([P, SO, VW], store_dt, tag="vv")
            if store_dt == F32:
                nc.vector.memset(vv[:, :, U:VW], 1.0)
            else:
                # memset can't write f32r/bf16-typed rounded values directly;
                # memset f32 then round via tensor_copy.
                ones_f32 = smalls.tile([P, SO, VW - U], F32, tag="ones_f32")
                nc.vector.memset(ones_f32[:], 1.0)
                nc.vector.tensor_copy(vv[:, :, U:VW], ones_f32[:])

            for sb in range(NSB):
                sl = slice(sb * SB, (sb + 1) * SB)
                for wname, dst in (("Wq", qT), ("Wk", kT)):
                    for uo in range(UO):
                        ps = ps_big.tile([P, SB], F32, tag="ps_big")
                        for fo in range(FO):
                            nc.tensor.matmul(
                                ps[:],
                                w_t[wname][:, fo, uo * P : (uo + 1) * P],
                                xT[:, fo, sl],
                                start=(fo == 0),
                                stop=(fo == FO - 1),
                            )
                        nc.scalar.activation(dst[:, uo, sl], ps[:], TANH)
                for so in range(sb * SB // P, (sb + 1) * SB // P):
                    ps = ps_v.tile([P, U], F32, tag="ps_v")
                    for fo in range(FO):
                        nc.tensor.matmul(
                            ps[:],
                            xT[:, fo, so * P : (so + 1) * P],
                            w_t["Wv"][:, fo, :],
                            start=(fo == 0),
                            stop=(fo == FO - 1),
                        )
                    nc.scalar.activation(vv[:, so, :U], ps[:], TANH)

            # ---- attention per query block ----
            for qb in range(NQB):
                qsl = slice(qb * QB, (qb + 1) * QB)
                ex = exps.tile([P, SO, QB], store_dt, tag="ex")
                for to in range(SO):
                    ps = ps_big.tile([P, QB], F32, tag="ps_big")
                    for uo in range(UO):
                        nc.tensor.matmul(
                            ps[:],
                            kT[:, uo, to * P : (to + 1) * P],
                            qT[:, uo, qsl],
                            start=(uo == 0),
                            stop=(uo == UO - 1),
                        )
                    nc.scalar.activation(ex[:, to, :], ps[:], EXP, scale=SCALE)
                for ss in range(QB // P):
                    s0 = qb * QB + ss * P
                    ps = ps_o.tile([P, VW], F32, tag="ps_o")
                    for to in range(SO):
                        nc.tensor.matmul(
                            ps[:],
                            ex[:, to, ss * P : (ss + 1) * P],
                            vv[:, to, :],
                            start=(to == 0),
                            stop=(to == SO - 1),
                        )
                    rec = recs.tile([P, 1], F32, tag="rec")
                    nc.vector.reciprocal(rec[:], ps[:, U : U + 1])
                    ot = evac.tile([P, U], F32, tag="ot")
                    nc.vector.tensor_scalar_mul(ot[:], ps[:, :U], rec[:])
                    nc.sync.dma_start(out_d[s0 : s0 + P, :], ot[:])

    if split_waits:
        _split_matmul_waits(nc)
    return nc


_NC_CACHE = {}


def _get_nc(cdt_name=CDT):
    if cdt_name not in _NC_CACHE:
        _NC_CACHE[cdt_name] = build_nc(cdt_name)
    return _NC_CACHE[cdt_name]


def make_in_maps(x, Wq, Wk, Wv):
    Wq = np.ascontiguousarray(np.asarray(Wq, dtype=np.float32))
    Wk = np.ascontiguousarray(np.asarray(Wk, dtype=np.float32))
    Wv = np.ascontiguousarray(np.asarray(Wv, dtype=np.float32))
    return [
        {
            "xT": np.ascontiguousarray(np.asarray(x[b], dtype=np.float32).T),
            "Wq": Wq,
            "Wk": Wk,
            "Wv": Wv,
        }
        for b in range(B)
    ]


def kernel(x, Wq, Wk, Wv):
    nc = _get_nc()
    in_maps = make_in_maps(x, Wq, Wk, Wv)
    res = run_bass_kernel_spmd(nc, in_maps, core_ids=list(range(B)))
    return np.stack(
        [np.asarray(res.results[i]["out"], dtype=np.float32) for i in range(B)],
        axis=0,
    )


# revision 38
# speedup vs baseline: 1.1475x; 1.1475x over previous
"""Trainium2 Bass kernel for nn_AttentionTanh (B=8, S=2048, F=1024, U=256).

Data-parallel over batch: each of the 8 NeuronCores computes the full
attention for one batch example. No collectives.

Per-core dataflow (all matmuls via TensorE, out = lhsT.T @ rhs):
  xT   [F, S]  (host-transposed input shard, F on partitions)
  QT   [u, s] = tanh(Wq.T @ x.T)  -> matmul(lhsT=Wq[f,u], rhs=xT[f,s])
  KT   [u, s] = tanh(Wk.T @ x.T)
  V    [s, u] = tanh(x @ Wv)      -> matmul(lhsT=xT[f,s], rhs=Wv[f,u])
                V gets a fused ones-column so the out-matmul also
                produces the softmax denominator (column U).
  eST  [t, q] = exp(scale * K q.T) -> matmul(lhsT=KT[u,t], rhs=QT[u,q])
                (tanh bounds scores to [-8, 8]; no max subtraction needed)
  out  [q, u] = (eST.T @ [V | 1]) row-normalized by its last column.
"""

import os
import sys

import numpy as np

for _p in ("/opt/trn_rl_repo", "/root/.axon_site/_ro/trn_rl_repo"):
    if os.path.isdir(_p) and _p not in sys.path:
        sys.path.append(_p)

import concourse.bass as bass
import concourse.mybir as mybir
import concourse.tile as tile
from concourse.bass_utils import run_bass_kernel_spmd

P = 128
B, S, F, U = 8, 2048, 1024, 256
FO, SO, UO = F // P, S // P, U // P  # 8, 16, 2
SB = 512                             # s-block width for DMA/projections
NSB = S // SB                        # 4
QB = 512                             # query-block width (free dim of eST)
NQB = S // QB                        # 4
SCALE = 1.0 / float(np.sqrt(F))      # 1/32
VW = U + 2                           # V plus fused ones columns (even width
                                     # keeps fp32r's free-count rules happy)
F32 = mybir.dt.float32

# Compute dtype for TensorE matmuls: "float32", "float32r", or "bfloat16".
CDT = "float32r"


def _split_matmul_waits(nc):
    """Walrus instruction structs have a single sem-wait slot (EventSemaphore
    has two). Peel excess waits onto NoOps (plain wait instructions on the
    same engine) inserted just before the overloaded instruction."""
    n = 0
    for bb in nc.m.functions[0].blocks:
        new_insts = []
        for inst in bb.instructions:
            cap = 2 if isinstance(inst, mybir.InstEventSemaphore) else 1
            if (
                inst.sync_info
                and inst.sync_info.on_wait
                and len(inst.sync_info.on_wait) > cap
            ):
                waits = list(inst.sync_info.on_wait)
                for w in waits[cap:]:
                    n += 1
                    nop = mybir.InstNoOp(name=f"I-xwait-{n}", ins=[], outs=[])
                    nop.engine = inst.engine
                    nop.sync_info = mybir.SyncInfo(on_wait=[w], on_update=[])
                    new_insts.append(nop)
                inst.sync_info.on_wait = waits[:cap]
            new_insts.append(inst)
        bb.instructions[:] = new_insts
    return n


def build_nc(cdt_name=CDT, split_waits=True):
    cdt = getattr(mybir.dt, cdt_name)
    store_dt = F32 if cdt == F32 else cdt
    # float32r shares the fp32 bit layout, so DRAM parameters can be declared
    # f32r directly and DMA'd without a rounding cast; bf16 still needs the
    # staged cast copy after DMA.
    in_dt = cdt if cdt == mybir.dt.float32r else F32
    needs_cast = store_dt != in_dt

    nc = bass.Bass()
    xT_d = nc.declare_dram_parameter("xT", [F, S], in_dt, isOutput=False)
    w_d = {
        k: nc.declare_dram_parameter(k, [F, U], in_dt, isOutput=False)
        for k in ("Wq", "Wk", "Wv")
    }
    out_d = nc.declare_dram_parameter("out", [S, U], F32, isOutput=True)

    TANH = mybir.ActivationFunctionType.Tanh
    EXP = mybir.ActivationFunctionType.Exp

    with tile.TileContext(nc) as tc:
        with (
            tc.tile_pool(name="wpool", bufs=1) as wpool,
            tc.tile_pool(name="xpool", bufs=1) as xpool,
            tc.tile_pool(name="xstage", bufs=2) as xstage,
            tc.tile_pool(name="qkv", bufs=1) as qkv,
            tc.tile_pool(
                name="exps", bufs=2 if store_dt == mybir.dt.bfloat16 else 1
            ) as exps,
            tc.tile_pool(name="smalls", bufs=1) as smalls,
            tc.tile_pool(name="recs", bufs=2) as recs,
            tc.tile_pool(name="evac", bufs=4) as evac,
            tc.tile_pool(name="ps_big", bufs=2, space="PSUM") as ps_big,
            tc.tile_pool(name="ps_v", bufs=2, space="PSUM") as ps_v,
            tc.tile_pool(name="ps_o", bufs=2, space="PSUM") as ps_o,
            tc.tile_pool(name="ps_d", bufs=1, space="PSUM") as ps_dp,
        ):
            # ---- input DMAs. All on the sync/SP queue: SP-issued DMAs fan
            # out over many SDMA engines, while scalar/gpsimd-issued DMAs
            # serialize on one engine (~3x slower — measured). Order puts Wq
            # then x-block 0 first so the first QT matmul can start early. ----
            xT = xpool.tile([P, FO, S], store_dt)
            xT_src = xT_d[:].rearrange("(fo fi) s -> fi fo s", fi=P)
            w_t = {}
            w_stage = {}
            for k in ("Wq", "Wk", "Wv"):
                if needs_cast:
                    wstg = xstage.tile(
                        [P, FO, SB], in_dt, tag="stage", name=f"wstg_{k}"
                    )
                    w_stage[k] = wstg[:, :, :U]
                    w_t[k] = wpool.tile([P, FO, U], store_dt, tag=f"{k}_c", name=f"w_{k}_c")
                else:
                    w_t[k] = wpool.tile([P, FO, U], in_dt, tag=f"{k}_in", name=f"w_{k}")
                    w_stage[k] = w_t[k]

            def dma_w(k):
                nc.sync.dma_start(
                    w_stage[k][:], w_d[k][:].rearrange("(fo fi) u -> fi fo u", fi=P)
                )
                if needs_cast:
                    nc.vector.tensor_copy(w_t[k][:], w_stage[k][:])

            x_stage = []

            def dma_x(sb):
                sl = slice(sb * SB, (sb + 1) * SB)
                if not needs_cast:
                    nc.sync.dma_start(xT[:, :, sl], xT_src[:, :, sl])
                else:
                    xs = xstage.tile([P, FO, SB], in_dt, tag="stage")
                    nc.sync.dma_start(xs[:], xT_src[:, :, sl])
                    nc.vector.tensor_copy(xT[:, :, sl], xs[:])

            dma_w("Wq")
            dma_x(0)
            dma_w("Wk")
            dma_w("Wv")
            for sb in range(1, NSB):
                dma_x(sb)

            # ---- PE warmup: junk matmuls on a zeroed tile keep the PE busy
            # while the x DMAs land, so HAM un-throttles before real work ----
            warm_f32 = smalls.tile([P, SB], F32, tag="warm_f32")
            nc.vector.memset(warm_f32[:], 0.0)
            if store_dt == F32:
                warm = warm_f32
            else:
                warm = smalls.tile([P, SB], store_dt, tag="warm")
                nc.vector.tensor_copy(warm[:], warm_f32[:])
            ps_w = ps_dp.tile([P, SB], F32, tag="ps_w")
            for _ in range(6):
                nc.tensor.matmul(
                    ps_w[:], warm[:, :P], warm[:], start=True, stop=True
                )

            # ---- projections (per s-block so PE starts as DMA lands) ----
            qT = qkv.tile([P, UO, S], store_dt, tag="qT")
            kT = qkv.tile([P, UO, S], store_dt, tag="kT")
            # V gets two fused ones-columns: the out-matmul then also produces
            # the softmax denominator (cols U:U+2; two columns keep fp32r's
            # even-free-count rule satisfied).
            vv = qkv.tile([P, SO, VW], store_dt, tag="vv")
            if store_dt == F32:
                nc.vector.memset(vv[:, :, U:VW], 1.0)
            else:
                # memset can't write f32r/bf16-typed rounded values directly;
                # memset f32 then round via tensor_copy.
                ones_f32 = smalls.tile([P, SO, VW - U], F32, tag="ones_f32")
                nc.vector.memset(ones_f32[:], 1.0)
                nc.vector.tensor_copy(vv[:, :, U:VW], ones_f32[:])

            for sb in range(NSB):
                sl = slice(sb * SB, (sb + 1) * SB)
                for wname, dst in (("Wq", qT), ("Wk", kT)):
                    for uo in range(UO):
                        ps = ps_big.tile([P, SB], F32, tag="ps_big")
                        for fo in range(FO):
                            nc.tensor.matmul(
                                ps[:],
                                w_t[wname][:, fo, uo * P : (uo + 1) * P],
                                xT[:, fo, sl],
                                start=(fo == 0),
                                stop=(fo == FO - 1),
                            )
                        nc.scalar.activation(dst[:, uo, sl], ps[:], TANH)
                for so in range(sb * SB // P, (sb + 1) * SB // P):
                    ps = ps_v.tile([P, U], F32, tag="ps_v")
                    for fo in range(FO):
                        nc.tensor.matmul(
                            ps[:],
                            xT[:, fo, so * P : (so + 1) * P],
                            w_t["Wv"][:, fo, :],
                            start=(fo == 0),
                            stop=(fo == FO - 1),
                        )
                    nc.scalar.activation(vv[:, so, :U], ps[:], TANH)

            # ---- attention per query block ----
            for qb in range(NQB):
                qsl = slice(qb * QB, (qb + 1) * QB)
                ex = exps.tile([P, SO, QB], store_dt, tag="ex")
                for to in range(SO):
                    ps = ps_big.tile([P, QB], F32, tag="ps_big")
                    for uo in range(UO):
                        nc.tensor.matmul(
                            ps[:],
                            kT[:, uo, to * P : (to + 1) * P],
                            qT[:, uo, qsl],
                            start=(uo == 0),
                            stop=(uo == UO - 1),
                        )
                    nc.scalar.activation(ex[:, to, :], ps[:], EXP, scale=SCALE)
                for ss in range(QB // P):
                    s0 = qb * QB + ss * P
                    ps = ps_o.tile([P, VW], F32, tag="ps_o")
                    for to in range(SO):
                        nc.tensor.matmul(
                            ps[:],
                            ex[:, to, ss * P : (ss + 1) * P],
                            vv[:, to, :],
                            start=(to == 0),
                            stop=(to == SO - 1),
                        )
                    rec = recs.tile([P, 1], F32, tag="rec")
                    nc.vector.reciprocal(rec[:], ps[:, U : U + 1])
                    ot = evac.tile([P, U], F32, tag="ot")
                    nc.vector.tensor_scalar_mul(ot[:], ps[:, :U], rec[:])
                    nc.sync.dma_start(out_d[s0 : s0 + P, :], ot[:])

    if split_waits:
        _split_matmul_waits(nc)
    return nc


_NC_CACHE = {}


def _get_nc(cdt_name=CDT):
    if cdt_name not in _NC_CACHE:
        _NC_CACHE[cdt_name] = build_nc(cdt_name)
    return _NC_CACHE[cdt_name]


def make_in_maps(x, Wq, Wk, Wv):
    Wq = np.ascontiguousarray(np.asarray(Wq, dtype=np.float32))
    Wk = np.ascontiguousarray(np.asarray(Wk, dtype=np.float32))
    Wv = np.ascontiguousarray(np.asarray(Wv, dtype=np.float32))
    return [
        {
            "xT": np.ascontiguousarray(np.asarray(x[b], dtype=np.float32).T),
            "Wq": Wq,
            "Wk": Wk,
            "Wv": Wv,
        }
        for b in range(B)
    ]


def kernel(x, Wq, Wk, Wv):
    nc = _get_nc()
    in_maps = make_in_maps(x, Wq, Wk, Wv)
    res = run_bass_kernel_spmd(nc, in_maps, core_ids=list(range(B)))
    return np.stack(
        [np.asarray(res.results[i]["out"], dtype=np.float32) for i in range(B)],
        axis=0,
    )


# revision 43
# speedup vs baseline: 1.1798x; 1.0282x over previous
"""Trainium2 Bass kernel for nn_AttentionTanh (B=8, S=2048, F=1024, U=256).

Data-parallel over batch: each of the 8 NeuronCores computes the full
attention for one batch example. No collectives.

Per-core dataflow (all matmuls via TensorE, out = lhsT.T @ rhs):
  xT   [F, S]  (host-transposed input shard, F on partitions)
  QT   [u, s] = tanh(Wq.T @ x.T)  -> matmul(lhsT=Wq[f,u], rhs=xT[f,s])
  KT   [u, s] = tanh(Wk.T @ x.T)
  V    [s, u] = tanh(x @ Wv)      -> matmul(lhsT=xT[f,s], rhs=Wv[f,u])
                V gets a fused ones-column so the out-matmul also
                produces the softmax denominator (column U).
  eST  [t, q] = exp(scale * K q.T) -> matmul(lhsT=KT[u,t], rhs=QT[u,q])
                (tanh bounds scores to [-8, 8]; no max subtraction needed)
  out  [q, u] = (eST.T @ [V | 1]) row-normalized by its last column.
"""

import os
import sys

import numpy as np

for _p in ("/opt/trn_rl_repo", "/root/.axon_site/_ro/trn_rl_repo"):
    if os.path.isdir(_p) and _p not in sys.path:
        sys.path.append(_p)

import concourse.bass as bass
import concourse.mybir as mybir
import concourse.tile as tile
from concourse.bass_utils import run_bass_kernel_spmd

P = 128
B, S, F, U = 8, 2048, 1024, 256
FO, SO, UO = F // P, S // P, U // P  # 8, 16, 2
SB = 512                             # s-block width for DMA/projections
NSB = S // SB                        # 4
QB = 512                             # query-block width (free dim of eST)
NQB = S // QB                        # 4
SCALE = 1.0 / float(np.sqrt(F))      # 1/32
VW = U + 2                           # V plus fused ones columns (even width
                                     # keeps fp32r's free-count rules happy)
F32 = mybir.dt.float32

# Compute dtype for TensorE matmuls: "float32", "float32r", or "bfloat16".
CDT = "float32r"


def _split_matmul_waits(nc):
    """Walrus instruction structs have a single sem-wait slot (EventSemaphore
    has two). Peel excess waits onto NoOps (plain wait instructions on the
    same engine) inserted just before the overloaded instruction."""
    n = 0
    for bb in nc.m.functions[0].blocks:
        new_insts = []
        for inst in bb.instructions:
            cap = 2 if isinstance(inst, mybir.InstEventSemaphore) else 1
            if (
                inst.sync_info
                and inst.sync_info.on_wait
                and len(inst.sync_info.on_wait) > cap
            ):
                waits = list(inst.sync_info.on_wait)
                for w in waits[cap:]:
                    n += 1
                    nop = mybir.InstNoOp(name=f"I-xwait-{n}", ins=[], outs=[])
                    nop.engine = inst.engine
                    nop.sync_info = mybir.SyncInfo(on_wait=[w], on_update=[])
                    new_insts.append(nop)
                inst.sync_info.on_wait = waits[:cap]
            new_insts.append(inst)
        bb.instructions[:] = new_insts
    return n


def build_nc(cdt_name=CDT, split_waits=True):
    cdt = getattr(mybir.dt, cdt_name)
    store_dt = F32 if cdt == F32 else cdt
    # float32r shares the fp32 bit layout, so DRAM parameters can be declared
    # f32r directly and DMA'd without a rounding cast; bf16 still needs the
    # staged cast copy after DMA.
    in_dt = cdt if cdt == mybir.dt.float32r else F32
    needs_cast = store_dt != in_dt

    nc = bass.Bass()
    # Host pre-swizzles inputs to SBUF-matching layouts so every DMA is one
    # long contiguous run per partition (16KB for x blocks, 8KB for weights).
    xT_d = nc.declare_dram_parameter("xT", [P, NSB, FO, SB], in_dt, isOutput=False)
    w_d = {
        k: nc.declare_dram_parameter(k, [P, FO, U], in_dt, isOutput=False)
        for k in ("Wq", "Wk", "Wv")
    }
    out_d = nc.declare_dram_parameter("out", [S, U], F32, isOutput=True)

    TANH = mybir.ActivationFunctionType.Tanh
    EXP = mybir.ActivationFunctionType.Exp

    with tile.TileContext(nc) as tc:
        with (
            tc.tile_pool(name="wpool", bufs=1) as wpool,
            tc.tile_pool(name="xpool", bufs=1) as xpool,
            tc.tile_pool(name="xstage", bufs=2) as xstage,
            tc.tile_pool(name="qkv", bufs=1) as qkv,
            tc.tile_pool(
                name="exps", bufs=2 if store_dt == mybir.dt.bfloat16 else 1
            ) as exps,
            tc.tile_pool(name="smalls", bufs=1) as smalls,
            tc.tile_pool(name="recs", bufs=2) as recs,
            tc.tile_pool(name="evac", bufs=4) as evac,
            tc.tile_pool(name="ps_big", bufs=2, space="PSUM") as ps_big,
            tc.tile_pool(name="ps_v", bufs=2, space="PSUM") as ps_v,
            tc.tile_pool(name="ps_o", bufs=2, space="PSUM") as ps_o,
            tc.tile_pool(name="ps_d", bufs=1, space="PSUM") as ps_dp,
        ):
            # ---- input DMAs. All on the sync/SP queue: SP-issued DMAs fan
            # out over many SDMA engines, while scalar/gpsimd-issued DMAs
            # serialize on one engine (~3x slower — measured). Order puts Wq
            # then x-block 0 first so the first QT matmul can start early. ----
            xT = xpool.tile([P, NSB, FO, SB], store_dt)
            w_t = {}
            w_stage = {}
            for k in ("Wq", "Wk", "Wv"):
                if needs_cast:
                    wstg = xstage.tile(
                        [P, FO, SB], in_dt, tag="stage", name=f"wstg_{k}"
                    )
                    w_stage[k] = wstg[:, :, :U]
                    w_t[k] = wpool.tile(
                        [P, FO, U], store_dt, tag=f"{k}_c", name=f"w_{k}_c"
                    )
                else:
                    w_t[k] = wpool.tile(
                        [P, FO, U], in_dt, tag=f"{k}_in", name=f"w_{k}"
                    )
                    w_stage[k] = w_t[k]

            def dma_w(k):
                nc.sync.dma_start(w_stage[k][:], w_d[k][:])
                if needs_cast:
                    nc.vector.tensor_copy(w_t[k][:], w_stage[k][:])

            def dma_x(sb):
                if not needs_cast:
                    nc.sync.dma_start(xT[:, sb, :, :], xT_d[:, sb, :, :])
                else:
                    xs = xstage.tile([P, FO, SB], in_dt, tag="stage")
                    nc.sync.dma_start(xs[:], xT_d[:, sb, :, :])
                    nc.vector.tensor_copy(xT[:, sb, :, :], xs[:])

            dma_w("Wq")
            dma_x(0)
            dma_w("Wk")
            dma_w("Wv")
            for sb in range(1, NSB):
                dma_x(sb)

            # ---- PE warmup: junk matmuls on a zeroed tile keep the PE busy
            # while the x DMAs land, so HAM un-throttles before real work ----
            warm_f32 = smalls.tile([P, SB], F32, tag="warm_f32")
            nc.vector.memset(warm_f32[:], 0.0)
            if store_dt == F32:
                warm = warm_f32
            else:
                warm = smalls.tile([P, SB], store_dt, tag="warm")
                nc.vector.tensor_copy(warm[:], warm_f32[:])
            ps_w = ps_dp.tile([P, SB], F32, tag="ps_w")
            for _ in range(6):
                nc.tensor.matmul(
                    ps_w[:], warm[:, :P], warm[:], start=True, stop=True
                )

            # ---- projections (per s-block so PE starts as DMA lands) ----
            qT = qkv.tile([P, UO, S], store_dt, tag="qT")
            kT = qkv.tile([P, UO, S], store_dt, tag="kT")
            # V gets two fused ones-columns: the out-matmul then also produces
            # the softmax denominator (cols U:U+2; two columns keep fp32r's
            # even-free-count rule satisfied).
            vv = qkv.tile([P, SO, VW], store_dt, tag="vv")
            if store_dt == F32:
                nc.vector.memset(vv[:, :, U:VW], 1.0)
            else:
                # memset can't write f32r/bf16-typed rounded values directly;
                # memset f32 then round via tensor_copy.
                ones_f32 = smalls.tile([P, SO, VW - U], F32, tag="ones_f32")
                nc.vector.memset(ones_f32[:], 1.0)
                nc.vector.tensor_copy(vv[:, :, U:VW], ones_f32[:])

            for sb in range(NSB):
                sl = slice(sb * SB, (sb + 1) * SB)
                for wname, dst in (("Wq", qT), ("Wk", kT)):
                    for uo in range(UO):
                        ps = ps_big.tile([P, SB], F32, tag="ps_big")
                        for fo in range(FO):
                            nc.tensor.matmul(
                                ps[:],
                                w_t[wname][:, fo, uo * P : (uo + 1) * P],
                                xT[:, sb, fo, :],
                                start=(fo == 0),
                                stop=(fo == FO - 1),
                            )
                        nc.scalar.activation(dst[:, uo, sl], ps[:], TANH)
                for so in range(sb * SB // P, (sb + 1) * SB // P):
                    si = (so % (SB // P)) * P
                    ps = ps_v.tile([P, U], F32, tag="ps_v")
                    for fo in range(FO):
                        nc.tensor.matmul(
                            ps[:],
                            xT[:, sb, fo, si : si + P],
                            w_t["Wv"][:, fo, :],
                            start=(fo == 0),
                            stop=(fo == FO - 1),
                        )
                    nc.scalar.activation(vv[:, so, :U], ps[:], TANH)

            # ---- attention per query block ----
            for qb in range(NQB):
                qsl = slice(qb * QB, (qb + 1) * QB)
                ex = exps.tile([P, SO, QB], store_dt, tag="ex")
                for to in range(SO):
                    ps = ps_big.tile([P, QB], F32, tag="ps_big")
                    for uo in range(UO):
                        nc.tensor.matmul(
                            ps[:],
                            kT[:, uo, to * P : (to + 1) * P],
                            qT[:, uo, qsl],
                            start=(uo == 0),
                            stop=(uo == UO - 1),
                        )
                    nc.scalar.activation(ex[:, to, :], ps[:], EXP, scale=SCALE)
                for ss in range(QB // P):
                    s0 = qb * QB + ss * P
                    ps = ps_o.tile([P, VW], F32, tag="ps_o")
                    for to in range(SO):
                        nc.tensor.matmul(
                            ps[:],
                            ex[:, to, ss * P : (ss + 1) * P],
                            vv[:, to, :],
                            start=(to == 0),
                            stop=(to == SO - 1),
                        )
                    rec = recs.tile([P, 1], F32, tag="rec")
                    nc.vector.reciprocal(rec[:], ps[:, U : U + 1])
                    ot = evac.tile([P, U], F32, tag="ot")
                    nc.vector.tensor_scalar_mul(ot[:], ps[:, :U], rec[:])
                    nc.sync.dma_start(out_d[s0 : s0 + P, :], ot[:])

    if split_waits:
        _split_matmul_waits(nc)
    return nc


_NC_CACHE = {}


def _get_nc(cdt_name=CDT):
    if cdt_name not in _NC_CACHE:
        _NC_CACHE[cdt_name] = build_nc(cdt_name)
    return _NC_CACHE[cdt_name]


def _swizzle_w(w):
    # [F, U] -> [fi, fo, u]: contiguous 8KB per partition row.
    w = np.asarray(w, dtype=np.float32)
    return np.ascontiguousarray(w.reshape(FO, P, U).transpose(1, 0, 2))


def _swizzle_x(xb):
    # [S, F] -> xT [fi, sb, fo, s]: each s-block DMA is one contiguous 16KB
    # run per partition.
    xT = np.asarray(xb, dtype=np.float32).T  # [F, S]
    return np.ascontiguousarray(
        xT.reshape(FO, P, NSB, SB).transpose(1, 2, 0, 3)
    )


def make_in_maps(x, Wq, Wk, Wv):
    Wq, Wk, Wv = _swizzle_w(Wq), _swizzle_w(Wk), _swizzle_w(Wv)
    return [
        {"xT": _swizzle_x(x[b]), "Wq": Wq, "Wk": Wk, "Wv": Wv}
        for b in range(B)
    ]


def kernel(x, Wq, Wk, Wv):
    nc = _get_nc()
    in_maps = make_in_maps(x, Wq, Wk, Wv)
    res = run_bass_kernel_spmd(nc, in_maps, core_ids=list(range(B)))
    return np.stack(
        [np.asarray(res.results[i]["out"], dtype=np.float32) for i in range(B)],
        axis=0,
    )


# revision 45
# speedup vs baseline: 1.1945x; 1.0124x over previous
"""Trainium2 Bass kernel for nn_AttentionTanh (B=8, S=2048, F=1024, U=256).

Data-parallel over batch: each of the 8 NeuronCores computes the full
attention for one batch example. No collectives.

Per-core dataflow (all matmuls via TensorE, out = lhsT.T @ rhs):
  xT   [F, S]  (host-swizzled input shard, F on partitions)
  QT   [u, s] = tanh(Wq.T @ x.T)  -> matmul(lhsT=Wq[f,u], rhs=xT[f,s])
  KT   [u, s] = tanh(Wk.T @ x.T)
  V    [s, u] = tanh(x @ Wv)      -> matmul(lhsT=xT[f,s], rhs=Wv[f,u])
                V gets two fused ones-columns so the out-matmul also
                produces the softmax denominator (cols U:U+2).
  eST  [t, q] = exp(scale * K.T q) -> matmul(lhsT=KT[u,t], rhs=QT[u,q])
                (tanh bounds scores to [-8, 8]; no max subtraction needed)
  out  [q, u] = (eST.T @ [V | 1 1]) row-normalized by column U.
"""

import os
import sys

import numpy as np

for _p in ("/opt/trn_rl_repo", "/root/.axon_site/_ro/trn_rl_repo"):
    if os.path.isdir(_p) and _p not in sys.path:
        sys.path.append(_p)

import concourse.bass as bass
import concourse.mybir as mybir
import concourse.tile as tile
from concourse.bass_utils import run_bass_kernel_spmd

P = 128
B, S, F, U = 8, 2048, 1024, 256
FO, SO, UO = F // P, S // P, U // P  # 8, 16, 2
SB = 512                             # s-block width for DMA/projections
NSB = S // SB                        # 4
QB = 512                             # query-block width (free dim of eST)
NQB = S // QB                        # 4
SCALE = 1.0 / float(np.sqrt(F))      # 1/32
VW = U + 2                           # V plus fused ones columns (even width
                                     # keeps fp32r's free-count rules happy)
F32 = mybir.dt.float32

# Compute dtype for TensorE matmuls: "float32", "float32r", or "bfloat16".
CDT = "float32r"


def _split_matmul_waits(nc):
    """Walrus instruction structs have a single sem-wait slot (EventSemaphore
    has two). Peel excess waits onto NoOps (plain wait instructions on the
    same engine) inserted just before the overloaded instruction."""
    n = 0
    for bb in nc.m.functions[0].blocks:
        new_insts = []
        for inst in bb.instructions:
            cap = 2 if isinstance(inst, mybir.InstEventSemaphore) else 1
            if (
                inst.sync_info
                and inst.sync_info.on_wait
                and len(inst.sync_info.on_wait) > cap
            ):
                waits = list(inst.sync_info.on_wait)
                for w in waits[cap:]:
                    n += 1
                    nop = mybir.InstNoOp(name=f"I-xwait-{n}", ins=[], outs=[])
                    nop.engine = inst.engine
                    nop.sync_info = mybir.SyncInfo(on_wait=[w], on_update=[])
                    new_insts.append(nop)
                inst.sync_info.on_wait = waits[:cap]
            new_insts.append(inst)
        bb.instructions[:] = new_insts
    return n


def build_nc(cdt_name=CDT, split_waits=True):
    cdt = getattr(mybir.dt, cdt_name)
    store_dt = F32 if cdt == F32 else cdt
    # float32r shares the fp32 bit layout, so DRAM parameters can be declared
    # f32r directly and DMA'd without a rounding cast; bf16 still needs the
    # staged cast copy after DMA.
    in_dt = cdt if cdt == mybir.dt.float32r else F32
    needs_cast = store_dt != in_dt

    nc = bass.Bass()
    # Host pre-swizzles inputs to SBUF-matching layouts so every DMA is one
    # long contiguous run per partition (16KB for x blocks, 8KB for weights).
    xT_d = nc.declare_dram_parameter("xT", [P, NSB, FO, SB], in_dt, isOutput=False)
    w_d = {
        k: nc.declare_dram_parameter(k, [P, FO, U], in_dt, isOutput=False)
        for k in ("Wq", "Wk", "Wv")
    }
    out_d = nc.declare_dram_parameter("out", [S, U], F32, isOutput=True)

    TANH = mybir.ActivationFunctionType.Tanh
    EXP = mybir.ActivationFunctionType.Exp

    with tile.TileContext(nc) as tc:
        with (
            tc.tile_pool(name="wpool", bufs=1) as wpool,
            tc.tile_pool(name="qkv", bufs=1) as qkv,
            tc.tile_pool(name="smalls", bufs=1) as smalls,
            tc.tile_pool(name="recs", bufs=2) as recs,
            tc.tile_pool(name="evac", bufs=4) as evac,
            tc.tile_pool(name="ps_big", bufs=2, space="PSUM") as ps_big,
            tc.tile_pool(name="ps_v", bufs=2, space="PSUM") as ps_v,
            tc.tile_pool(name="ps_o", bufs=2, space="PSUM") as ps_o,
            tc.tile_pool(name="ps_d", bufs=1, space="PSUM") as ps_dp,
        ):
            # ---- phase 1: loads + projections. xT lives only here; its
            # SBUF space is released to the exp tiles afterwards. ----
            with (
                tc.tile_pool(name="xpool", bufs=1) as xpool,
                tc.tile_pool(name="xstage", bufs=2) as xstage,
            ):
                # All DMAs ride the sync/SP queue: SP-issued DMAs fan out
                # over many SDMA engines, while scalar/gpsimd-issued DMAs
                # serialize on one engine (~3x slower — measured). Wq and
                # x-block 0 go first; block 0 is further split per fo chunk
                # so the first QT matmul starts after ~1.25MB, not 3MB.
                xT = xpool.tile([P, NSB, FO, SB], store_dt)
                w_t = {}
                w_stage = {}
                for k in ("Wq", "Wk", "Wv"):
                    if needs_cast:
                        wstg = xstage.tile(
                            [P, FO, SB], in_dt, tag="stage", name=f"wstg_{k}"
                        )
                        w_stage[k] = wstg[:, :, :U]
                        w_t[k] = wpool.tile(
                            [P, FO, U], store_dt, tag=f"{k}_c", name=f"w_{k}_c"
                        )
                    else:
                        w_t[k] = wpool.tile(
                            [P, FO, U], in_dt, tag=f"{k}_in", name=f"w_{k}"
                        )
                        w_stage[k] = w_t[k]

                def dma_w(k):
                    nc.sync.dma_start(w_stage[k][:], w_d[k][:])
                    if needs_cast:
                        nc.vector.tensor_copy(w_t[k][:], w_stage[k][:])

                def dma_x(sb, split=False):
                    if not needs_cast:
                        if split:
                            for fo in range(FO):
                                nc.sync.dma_start(
                                    xT[:, sb, fo, :], xT_d[:, sb, fo, :]
                                )
                        else:
                            nc.sync.dma_start(xT[:, sb, :, :], xT_d[:, sb, :, :])
                    else:
                        xs = xstage.tile([P, FO, SB], in_dt, tag="stage")
                        nc.sync.dma_start(xs[:], xT_d[:, sb, :, :])
                        nc.vector.tensor_copy(xT[:, sb, :, :], xs[:])

                dma_w("Wq")
                dma_x(0, split=True)
                dma_w("Wk")
                dma_w("Wv")
                for sb in range(1, NSB):
                    dma_x(sb)

                # PE warmup: junk matmuls on a zeroed tile keep the PE busy
                # while the x DMAs land, so HAM un-throttles before real work.
                warm_f32 = smalls.tile([P, SB], F32, tag="warm_f32")
                nc.vector.memset(warm_f32[:], 0.0)
                if store_dt == F32:
                    warm = warm_f32
                else:
                    warm = smalls.tile([P, SB], store_dt, tag="warm")
                    nc.vector.tensor_copy(warm[:], warm_f32[:])
                ps_w = ps_dp.tile([P, SB], F32, tag="ps_w")
                for _ in range(6):
                    nc.tensor.matmul(
                        ps_w[:], warm[:, :P], warm[:], start=True, stop=True
                    )

                # ---- projections (per s-block so PE starts as DMA lands) ----
                qT = qkv.tile([P, UO, S], store_dt, tag="qT")
                kT = qkv.tile([P, UO, S], store_dt, tag="kT")
                vv = qkv.tile([P, SO, VW], store_dt, tag="vv")
                if store_dt == F32:
                    nc.vector.memset(vv[:, :, U:VW], 1.0)
                else:
                    # memset can't write f32r/bf16-typed rounded values
                    # directly; memset f32 then round via tensor_copy.
                    ones_f32 = smalls.tile([P, SO, VW - U], F32, tag="ones_f32")
                    nc.vector.memset(ones_f32[:], 1.0)
                    nc.vector.tensor_copy(vv[:, :, U:VW], ones_f32[:])

                for sb in range(NSB):
                    sl = slice(sb * SB, (sb + 1) * SB)
                    for wname, dst in (("Wq", qT), ("Wk", kT)):
                        for uo in range(UO):
                            ps = ps_big.tile([P, SB], F32, tag="ps_big")
                            for fo in range(FO):
                                nc.tensor.matmul(
                                    ps[:],
                                    w_t[wname][:, fo, uo * P : (uo + 1) * P],
                                    xT[:, sb, fo, :],
                                    start=(fo == 0),
                                    stop=(fo == FO - 1),
                                )
                            nc.scalar.activation(dst[:, uo, sl], ps[:], TANH)
                    for so in range(sb * SB // P, (sb + 1) * SB // P):
                        si = (so % (SB // P)) * P
                        ps = ps_v.tile([P, U], F32, tag="ps_v")
                        for fo in range(FO):
                            nc.tensor.matmul(
                                ps[:],
                                xT[:, sb, fo, si : si + P],
                                w_t["Wv"][:, fo, :],
                                start=(fo == 0),
                                stop=(fo == FO - 1),
                            )
                        nc.scalar.activation(vv[:, so, :U], ps[:], TANH)

            # ---- phase 2: attention per query block ----
            with tc.tile_pool(name="exps", bufs=2) as exps:
                for qb in range(NQB):
                    qsl = slice(qb * QB, (qb + 1) * QB)
                    ex = exps.tile([P, SO, QB], store_dt, tag="ex")
                    for to in range(SO):
                        ps = ps_big.tile([P, QB], F32, tag="ps_big")
                        for uo in range(UO):
                            nc.tensor.matmul(
                                ps[:],
                                kT[:, uo, to * P : (to + 1) * P],
                                qT[:, uo, qsl],
                                start=(uo == 0),
                                stop=(uo == UO - 1),
                            )
                        nc.scalar.activation(ex[:, to, :], ps[:], EXP, scale=SCALE)
                    for ss in range(QB // P):
                        s0 = qb * QB + ss * P
                        ps = ps_o.tile([P, VW], F32, tag="ps_o")
                        for to in range(SO):
                            nc.tensor.matmul(
                                ps[:],
                                ex[:, to, ss * P : (ss + 1) * P],
                                vv[:, to, :],
                                start=(to == 0),
                                stop=(to == SO - 1),
                            )
                        rec = recs.tile([P, 1], F32, tag="rec")
                        nc.vector.reciprocal(rec[:], ps[:, U : U + 1])
                        ot = evac.tile([P, U], F32, tag="ot")
                        nc.vector.tensor_scalar_mul(ot[:], ps[:, :U], rec[:])
                        nc.sync.dma_start(out_d[s0 : s0 + P, :], ot[:])

    if split_waits:
        _split_matmul_waits(nc)
    return nc


_NC_CACHE = {}


def _get_nc(cdt_name=CDT):
    if cdt_name not in _NC_CACHE:
        _NC_CACHE[cdt_name] = build_nc(cdt_name)
    return _NC_CACHE[cdt_name]


def _swizzle_w(w):
    # [F, U] -> [fi, fo, u]: contiguous 8KB per partition row.
    w = np.asarray(w, dtype=np.float32)
    return np.ascontiguousarray(w.reshape(FO, P, U).transpose(1, 0, 2))


def _swizzle_x(xb):
    # [S, F] -> xT [fi, sb, fo, s]: each s-block DMA is one contiguous 16KB
    # run per partition.
    xT = np.asarray(xb, dtype=np.float32).T  # [F, S]
    return np.ascontiguousarray(
        xT.reshape(FO, P, NSB, SB).transpose(1, 2, 0, 3)
    )


def make_in_maps(x, Wq, Wk, Wv):
    Wq, Wk, Wv = _swizzle_w(Wq), _swizzle_w(Wk), _swizzle_w(Wv)
    return [
        {"xT": _swizzle_x(x[b]), "Wq": Wq, "Wk": Wk, "Wv": Wv}
        for b in range(B)
    ]


def kernel(x, Wq, Wk, Wv):
    nc = _get_nc()
    in_maps = make_in_maps(x, Wq, Wk, Wv)
    res = run_bass_kernel_spmd(nc, in_maps, core_ids=list(range(B)))
    return np.stack(
        [np.asarray(res.results[i]["out"], dtype=np.float32) for i in range(B)],
        axis=0,
    )


# revision 47
# speedup vs baseline: 1.2472x; 1.0441x over previous
"""Trainium2 Bass kernel for nn_AttentionTanh (B=8, S=2048, F=1024, U=256).

Data-parallel over batch: each of the 8 NeuronCores computes the full
attention for one batch example. No collectives.

Per-core dataflow (all matmuls via TensorE, out = lhsT.T @ rhs):
  xT   [F, S]  (host-swizzled input shard, F on partitions)
  QT   [u, s] = tanh(Wq.T @ x.T)  -> matmul(lhsT=Wq[f,u], rhs=xT[f,s])
  KT   [u, s] = tanh(Wk.T @ x.T)
  V    [s, u] = tanh(x @ Wv)      -> matmul(lhsT=xT[f,s], rhs=Wv[f,u])
                V gets two fused ones-columns so the out-matmul also
                produces the softmax denominator (cols U:U+2).
  eST  [t, q] = exp(scale * K.T q) -> matmul(lhsT=KT[u,t], rhs=QT[u,q])
                (tanh bounds scores to [-8, 8]; no max subtraction needed)
  out  [q, u] = (eST.T @ [V | 1 1]) row-normalized by column U.
"""

import os
import sys

import numpy as np

for _p in ("/opt/trn_rl_repo", "/root/.axon_site/_ro/trn_rl_repo"):
    if os.path.isdir(_p) and _p not in sys.path:
        sys.path.append(_p)

import concourse.bass as bass
import concourse.mybir as mybir
import concourse.tile as tile
from concourse.bass_utils import run_bass_kernel_spmd

P = 128
B, S, F, U = 8, 2048, 1024, 256
FO, SO, UO = F // P, S // P, U // P  # 8, 16, 2
SB = 512                             # s-block width for DMA/projections
NSB = S // SB                        # 4
QB = 512                             # query-block width (free dim of eST)
NQB = S // QB                        # 4
SCALE = 1.0 / float(np.sqrt(F))      # 1/32
VW = U + 2                           # V plus fused ones columns (even width
                                     # keeps fp32r's free-count rules happy)
F32 = mybir.dt.float32

# Compute dtype for TensorE matmuls: "float32", "float32r", or "bfloat16".
CDT = "float32r"


def _split_matmul_waits(nc):
    """Walrus instruction structs have a single sem-wait slot (EventSemaphore
    has two). Peel excess waits onto NoOps (plain wait instructions on the
    same engine) inserted just before the overloaded instruction."""
    n = 0
    for bb in nc.m.functions[0].blocks:
        new_insts = []
        for inst in bb.instructions:
            cap = 2 if isinstance(inst, mybir.InstEventSemaphore) else 1
            if (
                inst.sync_info
                and inst.sync_info.on_wait
                and len(inst.sync_info.on_wait) > cap
            ):
                waits = list(inst.sync_info.on_wait)
                for w in waits[cap:]:
                    n += 1
                    nop = mybir.InstNoOp(name=f"I-xwait-{n}", ins=[], outs=[])
                    nop.engine = inst.engine
                    nop.sync_info = mybir.SyncInfo(on_wait=[w], on_update=[])
                    new_insts.append(nop)
                inst.sync_info.on_wait = waits[:cap]
            new_insts.append(inst)
        bb.instructions[:] = new_insts
    return n


def build_nc(cdt_name=CDT, split_waits=True):
    cdt = getattr(mybir.dt, cdt_name)
    store_dt = F32 if cdt == F32 else cdt
    # float32r shares the fp32 bit layout, so DRAM parameters can be declared
    # f32r directly and DMA'd without a rounding cast; bf16 still needs the
    # staged cast copy after DMA.
    in_dt = cdt if cdt == mybir.dt.float32r else F32
    needs_cast = store_dt != in_dt

    nc = bass.Bass()
    # Host pre-swizzles inputs to SBUF-matching layouts so every DMA is one
    # long contiguous run per partition (16KB for x blocks, 8KB for weights).
    xT_d = nc.declare_dram_parameter("xT", [P, NSB, FO, SB], in_dt, isOutput=False)
    w_d = {
        k: nc.declare_dram_parameter(k, [P, UO, FO, P], in_dt, isOutput=False)
        for k in ("Wq", "Wk")
    }
    w_d["Wv"] = nc.declare_dram_parameter("Wv", [P, FO, U], in_dt, isOutput=False)
    out_d = nc.declare_dram_parameter("out", [S, U], F32, isOutput=True)

    TANH = mybir.ActivationFunctionType.Tanh
    EXP = mybir.ActivationFunctionType.Exp

    with tile.TileContext(nc) as tc:
        with (
            tc.tile_pool(name="wpool", bufs=1) as wpool,
            tc.tile_pool(name="qkv", bufs=1) as qkv,
            tc.tile_pool(name="smalls", bufs=1) as smalls,
            tc.tile_pool(name="recs", bufs=2) as recs,
            tc.tile_pool(name="evac", bufs=4) as evac,
            tc.tile_pool(name="ps_big", bufs=2, space="PSUM") as ps_big,
            tc.tile_pool(name="ps_v", bufs=2, space="PSUM") as ps_v,
            tc.tile_pool(name="ps_o", bufs=2, space="PSUM") as ps_o,
            tc.tile_pool(name="ps_d", bufs=1, space="PSUM") as ps_dp,
        ):
            # ---- phase 1: loads + projections. xT lives only here; its
            # SBUF space is released to the exp tiles afterwards. ----
            with (
                tc.tile_pool(name="xpool", bufs=1) as xpool,
                tc.tile_pool(name="xstage", bufs=2) as xstage,
            ):
                # All DMAs ride the sync/SP queue: SP-issued DMAs fan out
                # over many SDMA engines, while scalar/gpsimd-issued DMAs
                # serialize on one engine (~3x slower — measured). Wq and
                # x-block 0 go first; block 0 is further split per fo chunk
                # so the first QT matmul starts after ~1.25MB, not 3MB.
                xT = xpool.tile([P, NSB, FO, SB], store_dt)
                w_t = {}
                w_stage = {}
                for k, shape in (
                    ("Wq", [P, UO, FO, P]),
                    ("Wk", [P, UO, FO, P]),
                    ("Wv", [P, FO, U]),
                ):
                    if needs_cast:
                        w_stage[k] = xstage.tile(
                            shape, in_dt, tag=f"wstg_{k}", name=f"wstg_{k}"
                        )
                        w_t[k] = wpool.tile(
                            shape, store_dt, tag=f"{k}_c", name=f"w_{k}_c"
                        )
                    else:
                        w_t[k] = wpool.tile(
                            shape, in_dt, tag=f"{k}_in", name=f"w_{k}"
                        )
                        w_stage[k] = w_t[k]

                def dma_w(k, uo=None):
                    if uo is None:
                        nc.sync.dma_start(w_stage[k][:], w_d[k][:])
                    else:
                        nc.sync.dma_start(w_stage[k][:, uo], w_d[k][:, uo])
                    if needs_cast and (uo is None or uo == UO - 1):
                        nc.vector.tensor_copy(w_t[k][:], w_stage[k][:])

                def dma_x(sb, split=False):
                    if not needs_cast:
                        if split:
                            for fo in range(FO):
                                nc.sync.dma_start(
                                    xT[:, sb, fo, :], xT_d[:, sb, fo, :]
                                )
                        else:
                            nc.sync.dma_start(xT[:, sb, :, :], xT_d[:, sb, :, :])
                    else:
                        xs = xstage.tile([P, FO, SB], in_dt, tag="stage")
                        nc.sync.dma_start(xs[:], xT_d[:, sb, :, :])
                        nc.vector.tensor_copy(xT[:, sb, :, :], xs[:])

                # Byte-ordered so each consumer's data lands just in time:
                # Wq half 0 + x0 feed the first QT group; Wk halves arrive
                # before KT of block 0; Wv before V of block 0.
                dma_w("Wq", 0)
                dma_x(0, split=True)
                dma_w("Wq", 1)
                dma_w("Wk", 0)
                dma_w("Wk", 1)
                dma_w("Wv")
                for sb in range(1, NSB):
                    dma_x(sb)

                # PE warmup: junk matmuls on a zeroed tile keep the PE busy
                # while the x DMAs land, so HAM un-throttles before real work.
                warm_f32 = smalls.tile([P, SB], F32, tag="warm_f32")
                nc.vector.memset(warm_f32[:], 0.0)
                if store_dt == F32:
                    warm = warm_f32
                else:
                    warm = smalls.tile([P, SB], store_dt, tag="warm")
                    nc.vector.tensor_copy(warm[:], warm_f32[:])
                ps_w = ps_dp.tile([P, SB], F32, tag="ps_w")
                for _ in range(3):
                    nc.tensor.matmul(
                        ps_w[:], warm[:, :P], warm[:], start=True, stop=True
                    )

                # ---- projections (per s-block so PE starts as DMA lands) ----
                qT = qkv.tile([P, UO, S], store_dt, tag="qT")
                kT = qkv.tile([P, UO, S], store_dt, tag="kT")
                vv = qkv.tile([P, SO, VW], store_dt, tag="vv")
                if store_dt == F32:
                    nc.vector.memset(vv[:, :, U:VW], 1.0)
                else:
                    # memset can't write f32r/bf16-typed rounded values
                    # directly; memset f32 then round via tensor_copy.
                    ones_f32 = smalls.tile([P, SO, VW - U], F32, tag="ones_f32")
                    nc.vector.memset(ones_f32[:], 1.0)
                    nc.vector.tensor_copy(vv[:, :, U:VW], ones_f32[:])

                for sb in range(NSB):
                    sl = slice(sb * SB, (sb + 1) * SB)
                    for wname, dst in (("Wq", qT), ("Wk", kT)):
                        for uo in range(UO):
                            ps = ps_big.tile([P, SB], F32, tag="ps_big")
                            for fo in range(FO):
                                nc.tensor.matmul(
                                    ps[:],
                                    w_t[wname][:, uo, fo, :],
                                    xT[:, sb, fo, :],
                                    start=(fo == 0),
                                    stop=(fo == FO - 1),
                                )
                            nc.scalar.activation(dst[:, uo, sl], ps[:], TANH)
                    for so in range(sb * SB // P, (sb + 1) * SB // P):
                        si = (so % (SB // P)) * P
                        ps = ps_v.tile([P, U], F32, tag="ps_v")
                        for fo in range(FO):
                            nc.tensor.matmul(
                                ps[:],
                                xT[:, sb, fo, si : si + P],
                                w_t["Wv"][:, fo, :],
                                start=(fo == 0),
                                stop=(fo == FO - 1),
                            )
                        nc.scalar.activation(vv[:, so, :U], ps[:], TANH)

            # ---- phase 2: attention per query block ----
            with tc.tile_pool(name="exps", bufs=2) as exps:
                for qb in range(NQB):
                    qsl = slice(qb * QB, (qb + 1) * QB)
                    ex = exps.tile([P, SO, QB], store_dt, tag="ex")
                    for to in range(SO):
                        ps = ps_big.tile([P, QB], F32, tag="ps_big")
                        for uo in range(UO):
                            nc.tensor.matmul(
                                ps[:],
                                kT[:, uo, to * P : (to + 1) * P],
                                qT[:, uo, qsl],
                                start=(uo == 0),
                                stop=(uo == UO - 1),
                            )
                        nc.scalar.activation(ex[:, to, :], ps[:], EXP, scale=SCALE)
                    for ss in range(QB // P):
                        s0 = qb * QB + ss * P
                        ps = ps_o.tile([P, VW], F32, tag="ps_o")
                        for to in range(SO):
                            nc.tensor.matmul(
                                ps[:],
                                ex[:, to, ss * P : (ss + 1) * P],
                                vv[:, to, :],
                                start=(to == 0),
                                stop=(to == SO - 1),
                            )
                        rec = recs.tile([P, 1], F32, tag="rec")
                        nc.vector.reciprocal(rec[:], ps[:, U : U + 1])
                        ot = evac.tile([P, U], F32, tag="ot")
                        nc.vector.tensor_scalar_mul(ot[:], ps[:, :U], rec[:])
                        nc.sync.dma_start(out_d[s0 : s0 + P, :], ot[:])

    if split_waits:
        _split_matmul_waits(nc)
    return nc


_NC_CACHE = {}


def _get_nc(cdt_name=CDT):
    if cdt_name not in _NC_CACHE:
        _NC_CACHE[cdt_name] = build_nc(cdt_name)
    return _NC_CACHE[cdt_name]


def _swizzle_w(w):
    # [F, U] -> [fi, fo, u]: contiguous 8KB per partition row.
    w = np.asarray(w, dtype=np.float32)
    return np.ascontiguousarray(w.reshape(FO, P, U).transpose(1, 0, 2))


def _swizzle_w_halves(w):
    # [F, U] -> [fi, uo, fo, ui]: each uo half is one contiguous 4KB run
    # per partition, so it can be DMA'd independently.
    w = np.asarray(w, dtype=np.float32)
    return np.ascontiguousarray(
        w.reshape(FO, P, UO, P).transpose(1, 2, 0, 3)
    )


def _swizzle_x(xb):
    # [S, F] -> xT [fi, sb, fo, s]: each s-block DMA is one contiguous 16KB
    # run per partition.
    xT = np.asarray(xb, dtype=np.float32).T  # [F, S]
    return np.ascontiguousarray(
        xT.reshape(FO, P, NSB, SB).transpose(1, 2, 0, 3)
    )


def make_in_maps(x, Wq, Wk, Wv):
    Wq, Wk = _swizzle_w_halves(Wq), _swizzle_w_halves(Wk)
    Wv = _swizzle_w(Wv)
    return [
        {"xT": _swizzle_x(x[b]), "Wq": Wq, "Wk": Wk, "Wv": Wv}
        for b in range(B)
    ]


def kernel(x, Wq, Wk, Wv):
    nc = _get_nc()
    in_maps = make_in_maps(x, Wq, Wk, Wv)
    res = run_bass_kernel_spmd(nc, in_maps, core_ids=list(range(B)))
    return np.stack(
        [np.asarray(res.results[i]["out"], dtype=np.float32) for i in range(B)],
        axis=0,
    )
